# revision 30
# baseline (speedup 1.0000x reference)
"""Swin-style window attention kernel for 8 TRN2 NeuronCores.

Sharding: data-parallel over batch B=32 -> 4 images per core. No collectives.

Per-core dataflow (B_local=4 images, 384ch x 56x56, WS=7, 12 heads, d=32):
  stripe = (image b, window-row wr): 7x56 = 392 pixels = 8 windows.

  1. qkv matmul (bf16): 9 chunks of (128, 392) PSUM -> SBUF window-major
     (w, r, c) with windows PADDED to 64 cols (valid :49). Scale folded into wq
     on host; q-chunk copies on ScalarE, k/v on VectorE.
  2. QK^T (hg, hq, w): lhsT=K (32,49), rhs=Q (32,49) -> S^T into per-head PSUM
     bank sps_hq (128, 4wp, 64) at window-parity band 64*(w%2). Same-head MMs
     share a row group (serialize); different heads use different banks --
     never two row groups writing one (bank, partition-range).
  3. exp on ACT: one op per (hg, hq) over the full (128, 4, 49) bank ->
     es (128, 4wp, 4hq, 49) bf16, w-parity banded. Junk pad rows are finite
     (pads zeroed once per qkv tile) and never contracted.
  4. bias: one DVE multiply per hg: es *= exp(bias^T) (host-precomputed,
     band-replicated expb3), broadcast over window-pairs.
  5. V^T: DMA transpose of padded window-pairs (128,128) bf16 SBUF->SBUF.
  6. denominators: ones-stationary matmuls batched over window-pairs (N=196)
     into the shared ob bank cols 4:8; AV (hg, w, hq): lhsT=V^T slice (49,32)
     at band 64*(w%2), rhs=E^T (49,49) -> ob cols 0:4, out partitions
     64*(w%2)+32*(hq%2), bank hq//2.
  7. reciprocal + normalize: attn = O^T * (1/r) fused PSUM->SBUF (DVE),
     un-banding parities into attn (128, 3, 8, 49) bf16.
  8. proj + b_proj via ACT copy that un-permutes window-major -> raster.
"""

import os
import numpy as np
import ml_dtypes

import concourse.bass as bass
import concourse.tile as tile
from concourse import bacc, mybir
from concourse.bass_utils import run_bass_kernel_spmd

F32 = mybir.dt.float32
BF16 = mybir.dt.bfloat16

B_LOC = 4      # images per core
C = 384        # channels
H = W = 56
WS = 7         # window size
NH = 12        # heads
D = 32         # head dim
NW = 8         # windows per stripe (56/7)
NPIX = WS * W  # 392 pixels per stripe
WW = WS * WS   # 49
WP = 64        # padded window stride

_CACHE = {}
LAST_EXEC_NS = None


def _rel_index(ws):
    coords = np.stack(np.meshgrid(np.arange(ws), np.arange(ws), indexing='ij')).reshape(2, -1)
    rel = (coords[:, :, None] - coords[:, None, :]).transpose(1, 2, 0).astype(np.int64)
    rel[..., 0] += ws - 1
    rel[..., 1] += ws - 1
    rel[..., 0] *= 2 * ws - 1
    return rel.sum(-1)


def build_bass():
    nc = bacc.Bacc("TRN2", target_bir_lowering=False, debug=False, num_devices=8)

    x_d = nc.dram_tensor("x", [B_LOC, C, H, W], F32, kind="ExternalInput")
    wqkvT_d = nc.dram_tensor("wqkvT", [C, 3 * C], BF16, kind="ExternalInput")
    wprojT_d = nc.dram_tensor("wprojT", [C, C], BF16, kind="ExternalInput")
    expb3_d = nc.dram_tensor("expb3", [128, 3, 4, WW], BF16, kind="ExternalInput")
    bproj_d = nc.dram_tensor("bproj", [C], F32, kind="ExternalInput")
    out_d = nc.dram_tensor("out", [B_LOC, C, H, W], F32, kind="ExternalOutput")

    with tile.TileContext(nc) as tc:
        with (
            tc.tile_pool(name="singles", bufs=1) as singles,
            tc.tile_pool(name="xp", bufs=2) as xp,
            tc.tile_pool(name="xbp", bufs=2) as xbp,
            tc.tile_pool(name="qkvp", bufs=2) as qkvp,
            tc.tile_pool(name="ep", bufs=2) as ep,
            tc.tile_pool(name="vtp", bufs=3) as vtp,
            tc.tile_pool(name="rp", bufs=3) as rp,
            tc.tile_pool(name="ap_", bufs=2) as ap_,
            tc.tile_pool(name="yp", bufs=3) as yp,
            tc.tile_pool(name="mm_ps", bufs=2, space="PSUM") as mm_ps,
            tc.tile_pool(name="s_ps", bufs=1, space="PSUM") as s_ps,
            tc.tile_pool(name="o_ps", bufs=1, space="PSUM") as o_ps,
        ):
            # ---- preload constants ----
            wqkvT_sb = singles.tile([128, 3, 3 * C], BF16)
            nc.sync.dma_start(out=wqkvT_sb, in_=wqkvT_d.ap().rearrange("(kc p) m -> p kc m", p=128))
            wprojT_sb = singles.tile([128, 3, C], BF16)
            nc.sync.dma_start(out=wprojT_sb, in_=wprojT_d.ap().rearrange("(kc p) m -> p kc m", p=128))
            expb3_sb = singles.tile([128, 3, 4, WW], BF16)
            nc.sync.dma_start(out=expb3_sb, in_=expb3_d.ap())
            bproj_sb = singles.tile([128, 3], F32)
            nc.sync.dma_start(out=bproj_sb, in_=bproj_d.ap().rearrange("(oc p) -> p oc", p=128))
            ones_sb = singles.tile([128, 32], BF16)
            nc.vector.memset(ones_sb, 1.0)

            for b in range(B_LOC):
                for wr in range(8):
                    # ---- load x stripe, cast to bf16 ----
                    x_t = xp.tile([128, 3, NPIX], F32, tag="x")
                    for kc in range(3):
                        nc.sync.dma_start(
                            out=x_t[:, kc],
                            in_=x_d[b, kc * 128:(kc + 1) * 128, wr * WS:(wr + 1) * WS, :]
                            .rearrange("c r w -> c (r w)"),
                        )
                    xb_t = xbp.tile([128, 3, NPIX], BF16, tag="xb")
                    nc.gpsimd.tensor_copy(out=xb_t, in_=x_t)

                    # ---- qkv matmul: 9 chunks; window-major padded SBUF ----
                    q_sb = qkvp.tile([128, 3, NW, WP], BF16, tag="q")
                    k_sb = qkvp.tile([128, 3, NW, WP], BF16, tag="k")
                    v_sb = qkvp.tile([128, 3, NW, WP], BF16, tag="v")
                    # zero pad cols once per tile (keeps psum/es junk finite)
                    nc.gpsimd.memset(q_sb[:, :, :, WW:], 0.0)
                    nc.gpsimd.memset(k_sb[:, :, :, WW:], 0.0)
                    nc.gpsimd.memset(v_sb[:, :, :, WW:], 0.0)
                    dst = {0: q_sb, 1: k_sb, 2: v_sb}
                    for oc in range(9):
                        ps_full = mm_ps.tile([128, 512], F32, tag="mmps")
                        ps = ps_full[:, :NPIX]
                        for kc in range(3):
                            nc.tensor.matmul(
                                ps,
                                lhsT=wqkvT_sb[:, kc, oc * 128:(oc + 1) * 128],
                                rhs=xb_t[:, kc],
                                start=(kc == 0), stop=(kc == 2),
                            )
                        # raster (r w c) -> window-major (w r c), pad stays 0
                        src = ps.rearrange("p (r w c) -> p w r c", r=WS, w=NW, c=WS)
                        o = dst[oc // 3][:, oc % 3, :, :WW].rearrange("p w (r c) -> p w r c", r=WS)
                        if oc // 3 == 0:
                            nc.scalar.copy(out=o, in_=src)
                        else:
                            nc.vector.tensor_copy(out=o, in_=src)

                    attn_sb = ap_.tile([128, 3, NW, WW], BF16, tag="attn")
                    for hg in range(3):
                        # ---- QK^T into 4 per-head banks, w-parity bands ----
                        sps = [s_ps.tile([128, 4, WP], F32, tag=f"sps{i}", name=f"sps{i}") for i in range(4)]
                        for w in range(NW):
                            po = WP * (w % 2)
                            for hq in range(4):
                                nc.tensor.matmul(
                                    sps[hq][po:po + WP, w // 2, :WW],
                                    lhsT=k_sb[hq * D:(hq + 1) * D, hg, w, :],
                                    rhs=q_sb[hq * D:(hq + 1) * D, hg, w, :WW],
                                    tile_position=(hq * D, po),
                                )
                        # ---- V^T via DMA transpose of padded pairs ----
                        vts = []
                        for wp in range(4):
                            vt = vtp.tile([128, 128], BF16, tag=f"vt{wp}")
                            nc.sync.dma_start(
                                out=vt,
                                in_=v_sb[:, hg, 2 * wp:2 * wp + 2, :]
                                .rearrange("p a b -> p (a b)"),
                                transpose=True)
                            vts.append(vt)
                        # ---- exp: one ACT op per head bank ----
                        es = ep.tile([128, 4, 4, WW], BF16, tag="es")
                        for hq in range(4):
                            nc.scalar.activation(
                                out=es[:, :, hq, :], in_=sps[hq][:, :, :WW],
                                func=mybir.ActivationFunctionType.Exp,
                            )
                        # ---- bias multiply (one DVE op) ----
                        nc.vector.tensor_mul(
                            out=es, in0=es,
                            in1=expb3_sb[:, hg, None, :, :].to_broadcast((128, 4, 4, WW)),
                        )
                        # ---- ob banks: cols 0:4 AV out, cols 4:8 r ----
                        ob = [o_ps.tile([128, 8, WP], F32, tag=f"ob{i}", name=f"ob{i}") for i in range(2)]
                        for hq in range(4):
                            for par in range(2):
                                po = WP * par
                                co = po + D * (hq % 2)
                                nc.tensor.matmul(
                                    ob[hq // 2][co:co + D, 4:8, :]
                                    .rearrange("p a b -> p (a b)")[:, :4 * WW],
                                    lhsT=ones_sb[po:po + WW, :],
                                    rhs=es[po:po + WW, :, hq, :],
                                    tile_position=(po, co),
                                )
                        for w in range(NW):
                            po = WP * (w % 2)
                            for hq in range(4):
                                co = po + D * (hq % 2)
                                nc.tensor.matmul(
                                    ob[hq // 2][co:co + D, w // 2, :WW],
                                    lhsT=vts[w // 2][po:po + WW, hq * D:(hq + 1) * D],
                                    rhs=es[po:po + WW, w // 2, hq, :],
                                    tile_position=(po, co),
                                )
                        # ---- reciprocal + normalize ----
                        for x_ in range(2):
                            rinv = rp.tile([128, 4 * WW], F32, tag=f"rinv{x_}")
                            nc.vector.reciprocal(
                                out=rinv,
                                in_=ob[x_][:, 4:8, :].rearrange("p a b -> p (a b)")[:, :4 * WW])
                            for par in range(2):
                                po = WP * par
                                nc.vector.tensor_mul(
                                    out=attn_sb[64 * x_:64 * x_ + 64, hg]
                                    .rearrange("p (b a) n -> p b a n", b=4)[:, :, par, :],
                                    in0=ob[x_][po:po + 64, 0:4, :WW],
                                    in1=rinv.rearrange("p (a b) -> p a b", a=4)[po:po + 64],
                                )

                    # ---- proj + bias, un-permute to raster, DMA out ----
                    for oc in range(3):
                        yps_full = mm_ps.tile([128, 512], F32, tag="mmps")
                        yps = yps_full[:, :NPIX]
                        for kc in range(3):
                            nc.tensor.matmul(
                                yps,
                                lhsT=wprojT_sb[:, kc, oc * 128:(oc + 1) * 128],
                                rhs=attn_sb[:, kc],
                                start=(kc == 0), stop=(kc == 2),
                            )
                        y_sb = yp.tile([128, NPIX], F32, tag="y")
                        nc.scalar.activation(
                            out=y_sb.rearrange("p (r w c) -> p r w c", r=WS, w=NW),
                            in_=yps.rearrange("p (w r c) -> p r w c", w=NW, r=WS, c=WS),
                            func=mybir.ActivationFunctionType.Identity,
                            bias=bproj_sb[:, oc:oc + 1],
                        )
                        nc.sync.dma_start(
                            out=out_d[b, oc * 128:(oc + 1) * 128, wr * WS:(wr + 1) * WS, :]
                            .rearrange("c r w -> c (r w)"),
                            in_=y_sb,
                        )
    nc.compile()
    return nc


def host_prep(w_qkv, bias_table, w_proj, b_proj):
    scale = D ** -0.5
    wq = w_qkv[0:C] * scale
    wqkvT = np.ascontiguousarray(
        np.concatenate([wq, w_qkv[C:2 * C], w_qkv[2 * C:]], 0).T
    ).astype(ml_dtypes.bfloat16)
    wprojT = np.ascontiguousarray(w_proj.T).astype(ml_dtypes.bfloat16)
    rel = _rel_index(WS)
    bias = bias_table[rel.reshape(-1)].reshape(WW, WW, NH)  # [n, m, h]
    expbT = np.exp(bias.astype(np.float64)).transpose(1, 2, 0)  # [m, h, n]
    # band-replicated: rows 0:49 and 64:113 = expbT, pad rows zero
    expb3 = np.zeros((128, 3, 4, WW), np.float64)
    for hg in range(3):
        for hq in range(4):
            expb3[0:WW, hg, hq, :] = expbT[:, 4 * hg + hq, :]
            expb3[64:64 + WW, hg, hq, :] = expbT[:, 4 * hg + hq, :]
    return (wqkvT, wprojT, expb3.astype(ml_dtypes.bfloat16),
            np.ascontiguousarray(b_proj, dtype=np.float32))


def kernel(x, w_qkv, bias_table, w_proj, b_proj):
    global LAST_EXEC_NS
    x = np.ascontiguousarray(x, dtype=np.float32)
    wqkvT, wprojT, expb3, bproj = host_prep(
        np.asarray(w_qkv, np.float32), np.asarray(bias_table, np.float32),
        np.asarray(w_proj, np.float32), np.asarray(b_proj, np.float32))

    if "nc" not in _CACHE:
        _CACHE["nc"] = build_bass()
    nc = _CACHE["nc"]

    in_maps = []
    for i in range(8):
        in_maps.append({
            "x": x[B_LOC * i:B_LOC * (i + 1)],
            "wqkvT": wqkvT, "wprojT": wprojT, "expb3": expb3, "bproj": bproj,
        })
    res = run_bass_kernel_spmd(nc, in_maps, core_ids=list(range(8)), trace=False)
    LAST_EXEC_NS = res.exec_time_ns
    out = np.concatenate([res.results[i]["out"] for i in range(8)], axis=0)
    return out


# revision 31
# speedup vs baseline: 1.0120x; 1.0120x over previous
"""Swin-style window attention kernel for 8 TRN2 NeuronCores.

Sharding: data-parallel over batch B=32 -> 4 images per core. No collectives.

Per-core dataflow (B_local=4 images, 384ch x 56x56, WS=7, 12 heads, d=32):
  stripe = (image b, window-row wr): 7x56 = 392 pixels = 8 windows.

  1. qkv matmul (bf16): 9 chunks of (128, 392) PSUM -> SBUF window-major
     (w, r, c) with windows PADDED to 64 cols (valid :49). Scale folded into wq
     on host; q-chunk copies on ScalarE, k/v on VectorE.
  2. QK^T (hg, hq, w): lhsT=K (32,49), rhs=Q (32,49) -> S^T into per-head PSUM
     bank sps_hq (128, 4wp, 64) at window-parity band 64*(w%2). Same-head MMs
     share a row group (serialize); different heads use different banks --
     never two row groups writing one (bank, partition-range).
  3. exp on ACT: one op per (hg, hq) over the full (128, 4, 49) bank ->
     es (128, 4wp, 4hq, 49) bf16, w-parity banded. Junk pad rows are finite
     (pads zeroed once per qkv tile) and never contracted.
  4. bias: one DVE multiply per hg: es *= exp(bias^T) (host-precomputed,
     band-replicated expb3), broadcast over window-pairs.
  5. V^T: DMA transpose of padded window-pairs (128,128) bf16 SBUF->SBUF.
  6. denominators: ones-stationary matmuls batched over window-pairs (N=196)
     into the shared ob bank cols 4:8; AV (hg, w, hq): lhsT=V^T slice (49,32)
     at band 64*(w%2), rhs=E^T (49,49) -> ob cols 0:4, out partitions
     64*(w%2)+32*(hq%2), bank hq//2.
  7. reciprocal + normalize: attn = O^T * (1/r) fused PSUM->SBUF (DVE),
     un-banding parities into attn (128, 3, 8, 49) bf16.
  8. proj + b_proj via ACT copy that un-permutes window-major -> raster.
"""

import os
import numpy as np
import ml_dtypes

import concourse.bass as bass
import concourse.tile as tile
from concourse import bacc, mybir
from concourse.bass_utils import run_bass_kernel_spmd

F32 = mybir.dt.float32
BF16 = mybir.dt.bfloat16

B_LOC = 4      # images per core
C = 384        # channels
H = W = 56
WS = 7         # window size
NH = 12        # heads
D = 32         # head dim
NW = 8         # windows per stripe (56/7)
NPIX = WS * W  # 392 pixels per stripe
WW = WS * WS   # 49
WP = 64        # padded window stride

_CACHE = {}
LAST_EXEC_NS = None


def _rel_index(ws):
    coords = np.stack(np.meshgrid(np.arange(ws), np.arange(ws), indexing='ij')).reshape(2, -1)
    rel = (coords[:, :, None] - coords[:, None, :]).transpose(1, 2, 0).astype(np.int64)
    rel[..., 0] += ws - 1
    rel[..., 1] += ws - 1
    rel[..., 0] *= 2 * ws - 1
    return rel.sum(-1)


def build_bass():
    nc = bacc.Bacc("TRN2", target_bir_lowering=False, debug=False, num_devices=8)

    x_d = nc.dram_tensor("x", [B_LOC, C, H, W], F32, kind="ExternalInput")
    wqkvT_d = nc.dram_tensor("wqkvT", [C, 3 * C], BF16, kind="ExternalInput")
    wprojT_d = nc.dram_tensor("wprojT", [C, C], BF16, kind="ExternalInput")
    expb3_d = nc.dram_tensor("expb3", [128, 3, 4, WW], BF16, kind="ExternalInput")
    bproj_d = nc.dram_tensor("bproj", [C], F32, kind="ExternalInput")
    out_d = nc.dram_tensor("out", [B_LOC, C, H, W], F32, kind="ExternalOutput")

    with tile.TileContext(nc) as tc:
        with (
            tc.tile_pool(name="singles", bufs=1) as singles,
            tc.tile_pool(name="xp", bufs=2) as xp,
            tc.tile_pool(name="xbp", bufs=2) as xbp,
            tc.tile_pool(name="qkvp", bufs=2) as qkvp,
            tc.tile_pool(name="ep", bufs=2) as ep,
            tc.tile_pool(name="vtp", bufs=3) as vtp,
            tc.tile_pool(name="rp", bufs=3) as rp,
            tc.tile_pool(name="ap_", bufs=2) as ap_,
            tc.tile_pool(name="yp", bufs=3) as yp,
            tc.tile_pool(name="mm_ps", bufs=2, space="PSUM") as mm_ps,
            tc.tile_pool(name="s_ps", bufs=1, space="PSUM") as s_ps,
            tc.tile_pool(name="o_ps", bufs=1, space="PSUM") as o_ps,
        ):
            # ---- preload constants ----
            wqkvT_sb = singles.tile([128, 3, 3 * C], BF16)
            nc.sync.dma_start(out=wqkvT_sb, in_=wqkvT_d.ap().rearrange("(kc p) m -> p kc m", p=128))
            wprojT_sb = singles.tile([128, 3, C], BF16)
            nc.sync.dma_start(out=wprojT_sb, in_=wprojT_d.ap().rearrange("(kc p) m -> p kc m", p=128))
            expb3_sb = singles.tile([128, 3, 4, WW], BF16)
            nc.sync.dma_start(out=expb3_sb, in_=expb3_d.ap())
            bproj_sb = singles.tile([128, 3], F32)
            nc.sync.dma_start(out=bproj_sb, in_=bproj_d.ap().rearrange("(oc p) -> p oc", p=128))
            ones_sb = singles.tile([128, 32], BF16)
            nc.vector.memset(ones_sb, 1.0)

            for b in range(B_LOC):
                for wr in range(8):
                    # ---- load x stripe, cast to bf16 ----
                    x_t = xp.tile([128, 3, NPIX], F32, tag="x")
                    for kc in range(3):
                        nc.sync.dma_start(
                            out=x_t[:, kc],
                            in_=x_d[b, kc * 128:(kc + 1) * 128, wr * WS:(wr + 1) * WS, :]
                            .rearrange("c r w -> c (r w)"),
                        )
                    xb_t = xbp.tile([128, 3, NPIX], BF16, tag="xb")
                    nc.gpsimd.tensor_copy(out=xb_t, in_=x_t)

                    # ---- qkv matmul: 9 chunks; window-major padded SBUF ----
                    q_sb = qkvp.tile([128, 3, NW, WP], BF16, tag="q")
                    k_sb = qkvp.tile([128, 3, NW, WP], BF16, tag="k")
                    v_sb = qkvp.tile([128, 3, NW, WP], BF16, tag="v")
                    # zero pad cols once per tile (keeps psum/es junk finite)
                    nc.gpsimd.memset(q_sb[:, :, :, WW:], 0.0)
                    nc.gpsimd.memset(k_sb[:, :, :, WW:], 0.0)
                    nc.gpsimd.memset(v_sb[:, :, :, WW:], 0.0)
                    dst = {0: q_sb, 1: k_sb, 2: v_sb}
                    for oc in (0, 3, 6, 1, 4, 7, 2, 5, 8):
                        ps_full = mm_ps.tile([128, 512], F32, tag="mmps")
                        ps = ps_full[:, :NPIX]
                        for kc in range(3):
                            nc.tensor.matmul(
                                ps,
                                lhsT=wqkvT_sb[:, kc, oc * 128:(oc + 1) * 128],
                                rhs=xb_t[:, kc],
                                start=(kc == 0), stop=(kc == 2),
                            )
                        # raster (r w c) -> window-major (w r c), pad stays 0
                        src = ps.rearrange("p (r w c) -> p w r c", r=WS, w=NW, c=WS)
                        o = dst[oc // 3][:, oc % 3, :, :WW].rearrange("p w (r c) -> p w r c", r=WS)
                        if oc // 3 == 0:
                            nc.scalar.copy(out=o, in_=src)
                        else:
                            nc.vector.tensor_copy(out=o, in_=src)

                    attn_sb = ap_.tile([128, 3, NW, WW], BF16, tag="attn")
                    for hg in range(3):
                        # ---- QK^T into 4 per-head banks, w-parity bands ----
                        sps = [s_ps.tile([128, 4, WP], F32, tag=f"sps{i}", name=f"sps{i}") for i in range(4)]
                        for w in range(NW):
                            po = WP * (w % 2)
                            for hq in range(4):
                                nc.tensor.matmul(
                                    sps[hq][po:po + WP, w // 2, :WW],
                                    lhsT=k_sb[hq * D:(hq + 1) * D, hg, w, :],
                                    rhs=q_sb[hq * D:(hq + 1) * D, hg, w, :WW],
                                    tile_position=(hq * D, po),
                                )
                        # ---- V^T via DMA transpose of padded pairs ----
                        vts = []
                        for wp in range(4):
                            vt = vtp.tile([128, 128], BF16, tag=f"vt{wp}")
                            nc.sync.dma_start(
                                out=vt,
                                in_=v_sb[:, hg, 2 * wp:2 * wp + 2, :]
                                .rearrange("p a b -> p (a b)"),
                                transpose=True)
                            vts.append(vt)
                        # ---- exp: one ACT op per head bank ----
                        es = ep.tile([128, 4, 4, WW], BF16, tag="es")
                        for hq in range(4):
                            nc.scalar.activation(
                                out=es[:, :, hq, :], in_=sps[hq][:, :, :WW],
                                func=mybir.ActivationFunctionType.Exp,
                            )
                        # ---- bias multiply (one DVE op) ----
                        nc.vector.tensor_mul(
                            out=es, in0=es,
                            in1=expb3_sb[:, hg, None, :, :].to_broadcast((128, 4, 4, WW)),
                        )
                        # ---- ob banks: cols 0:4 AV out, cols 4:8 r ----
                        ob = [o_ps.tile([128, 8, WP], F32, tag=f"ob{i}", name=f"ob{i}") for i in range(2)]
                        for hq in range(4):
                            for par in range(2):
                                po = WP * par
                                co = po + D * (hq % 2)
                                nc.tensor.matmul(
                                    ob[hq // 2][co:co + D, 4:8, :]
                                    .rearrange("p a b -> p (a b)")[:, :4 * WW],
                                    lhsT=ones_sb[po:po + WW, :],
                                    rhs=es[po:po + WW, :, hq, :],
                                    tile_position=(po, co),
                                )
                        for w in range(NW):
                            po = WP * (w % 2)
                            for hq in range(4):
                                co = po + D * (hq % 2)
                                nc.tensor.matmul(
                                    ob[hq // 2][co:co + D, w // 2, :WW],
                                    lhsT=vts[w // 2][po:po + WW, hq * D:(hq + 1) * D],
                                    rhs=es[po:po + WW, w // 2, hq, :],
                                    tile_position=(po, co),
                                )
                        # ---- reciprocal + normalize ----
                        for x_ in range(2):
                            rinv = rp.tile([128, 4 * WW], F32, tag=f"rinv{x_}")
                            nc.vector.reciprocal(
                                out=rinv,
                                in_=ob[x_][:, 4:8, :].rearrange("p a b -> p (a b)")[:, :4 * WW])
                            for par in range(2):
                                po = WP * par
                                nc.vector.tensor_mul(
                                    out=attn_sb[64 * x_:64 * x_ + 64, hg]
                                    .rearrange("p (b a) n -> p b a n", b=4)[:, :, par, :],
                                    in0=ob[x_][po:po + 64, 0:4, :WW],
                                    in1=rinv.rearrange("p (a b) -> p a b", a=4)[po:po + 64],
                                )

                    # ---- proj + bias, un-permute to raster, DMA out ----
                    for oc in range(3):
                        yps_full = mm_ps.tile([128, 512], F32, tag="mmps")
                        yps = yps_full[:, :NPIX]
                        for kc in range(3):
                            nc.tensor.matmul(
                                yps,
                                lhsT=wprojT_sb[:, kc, oc * 128:(oc + 1) * 128],
                                rhs=attn_sb[:, kc],
                                start=(kc == 0), stop=(kc == 2),
                            )
                        y_sb = yp.tile([128, NPIX], F32, tag="y")
                        nc.scalar.activation(
                            out=y_sb.rearrange("p (r w c) -> p r w c", r=WS, w=NW),
                            in_=yps.rearrange("p (w r c) -> p r w c", w=NW, r=WS, c=WS),
                            func=mybir.ActivationFunctionType.Identity,
                            bias=bproj_sb[:, oc:oc + 1],
                        )
                        nc.sync.dma_start(
                            out=out_d[b, oc * 128:(oc + 1) * 128, wr * WS:(wr + 1) * WS, :]
                            .rearrange("c r w -> c (r w)"),
                            in_=y_sb,
                        )
    nc.compile()
    return nc


def host_prep(w_qkv, bias_table, w_proj, b_proj):
    scale = D ** -0.5
    wq = w_qkv[0:C] * scale
    wqkvT = np.ascontiguousarray(
        np.concatenate([wq, w_qkv[C:2 * C], w_qkv[2 * C:]], 0).T
    ).astype(ml_dtypes.bfloat16)
    wprojT = np.ascontiguousarray(w_proj.T).astype(ml_dtypes.bfloat16)
    rel = _rel_index(WS)
    bias = bias_table[rel.reshape(-1)].reshape(WW, WW, NH)  # [n, m, h]
    expbT = np.exp(bias.astype(np.float64)).transpose(1, 2, 0)  # [m, h, n]
    # band-replicated: rows 0:49 and 64:113 = expbT, pad rows zero
    expb3 = np.zeros((128, 3, 4, WW), np.float64)
    for hg in range(3):
        for hq in range(4):
            expb3[0:WW, hg, hq, :] = expbT[:, 4 * hg + hq, :]
            expb3[64:64 + WW, hg, hq, :] = expbT[:, 4 * hg + hq, :]
    return (wqkvT, wprojT, expb3.astype(ml_dtypes.bfloat16),
            np.ascontiguousarray(b_proj, dtype=np.float32))


def kernel(x, w_qkv, bias_table, w_proj, b_proj):
    global LAST_EXEC_NS
    x = np.ascontiguousarray(x, dtype=np.float32)
    wqkvT, wprojT, expb3, bproj = host_prep(
        np.asarray(w_qkv, np.float32), np.asarray(bias_table, np.float32),
        np.asarray(w_proj, np.float32), np.asarray(b_proj, np.float32))

    if "nc" not in _CACHE:
        _CACHE["nc"] = build_bass()
    nc = _CACHE["nc"]

    in_maps = []
    for i in range(8):
        in_maps.append({
            "x": x[B_LOC * i:B_LOC * (i + 1)],
            "wqkvT": wqkvT, "wprojT": wprojT, "expb3": expb3, "bproj": bproj,
        })
    res = run_bass_kernel_spmd(nc, in_maps, core_ids=list(range(8)), trace=False)
    LAST_EXEC_NS = res.exec_time_ns
    out = np.concatenate([res.results[i]["out"] for i in range(8)], axis=0)
    return out
